# revision 4
# baseline (speedup 1.0000x reference)
"""Causal attention kernel for 8 Trainium2 NeuronCores.

Problem: x[4,2048,1024] fp32, Wq/Wk/Wv[1024,1024] fp32 (nn.Linear: y = x @ W.T),
single-head causal attention, softmax(QK^T/sqrt(D)) @ V.

Sharding: 2 cores per batch; within a batch, queries are split by row PARITY
(core h takes global rows s with s % 2 == h). This makes causal work exactly
balanced across the pair and keeps one uniform SPMD program (per-core
differences are pure data: xqT slice + causal mask tiles).

Device layout choices (PE matmul computes out = lhsT.T @ rhs, contraction over
the 128-partition dim):
  - host passes x^T and W^T so the contraction dim d lands on partitions with
    zero on-device transposes
  - Kt[e,k], Qt[e,q] produced directly by the projections (e on partitions)
  - scores computed transposed St[k,q]; softmax denominator done with a
    ones-vector matmul (PSUM accumulates in fp32); no max-subtraction needed
    (logits are bounded ~|2.5|)
  - exp on ScalarE writes P^T in bf16; AV matmul in bf16 (fp32 PSUM accumulate)
  - projections and scores run in float32r (full PE rate, ~1.5e-4 rel err)
"""

import numpy as np

B, S, D, P = 4, 2048, 1024, 128
NQ = S // 2          # queries per core (parity split)
QT = 256             # score-tile width in (core-local) query dim
NEG = -1e30
N_CORES = 8

_cache = {}


def _build():
    import concourse.mybir as mybir
    import concourse.tile as tile
    from concourse import bacc

    fr = mybir.dt.float32r
    f32 = mybir.dt.float32
    bf = mybir.dt.bfloat16

    nc = bacc.Bacc()

    xT = nc.dram_tensor("xT", [D, S], fr, kind="ExternalInput")
    xqT = nc.dram_tensor("xqT", [D, NQ], fr, kind="ExternalInput")
    wqT = nc.dram_tensor("wqT", [D, D], fr, kind="ExternalInput")
    wkT = nc.dram_tensor("wkT", [D, D], fr, kind="ExternalInput")
    wvT = nc.dram_tensor("wvT", [D, D], fr, kind="ExternalInput")
    masks = nc.dram_tensor("masks", [4, P, QT], f32, kind="ExternalInput")
    out = nc.dram_tensor("out", [NQ, D], f32, kind="ExternalOutput")

    xT3 = xT.ap().rearrange("(do di) s -> di do s", di=P)
    xq3 = xqT.ap().rearrange("(do di) s -> di do s", di=P)
    wq3 = wqT.ap().rearrange("(do di) e -> di do e", di=P)
    wk3 = wkT.ap().rearrange("(do di) e -> di do e", di=P)
    wv3 = wvT.ap().rearrange("(do di) e -> di do e", di=P)
    out_ap = out.ap()
    masks_ap = masks.ap()

    EXP = mybir.ActivationFunctionType.Exp
    SCALE = 1.0 / np.sqrt(np.float32(D))

    with tile.TileContext(nc) as tc:
        with (
            tc.tile_pool(name="const", bufs=1) as const_pool,
            tc.tile_pool(name="prod", bufs=1) as prod,
        ):
            mask_sb = const_pool.tile([P, 4, QT], f32)
            for m in range(4):
                nc.sync.dma_start(mask_sb[:, m, :], masks_ap[m])
            ones_sb = const_pool.tile([P, 1], bf)
            nc.vector.memset(ones_sb[:], 1.0)

            # persistent per-core products, slabbed for fine-grained deps
            kt_slabs = [prod.tile([P, 8, 512], fr, tag=f"kt{s}", name=f"kt{s}") for s in range(4)]
            v_slabs = [prod.tile([P, 4, D], bf, tag=f"v{s}", name=f"v{s}") for s in range(4)]
            qt = prod.tile([P, 8, NQ], fr, tag="qt")

            # ---- phase AB: K^T and V projections (keys 0..2047) ----
            with (
                tc.tile_pool(name="wv", bufs=1) as wv_pool,
                tc.tile_pool(name="xt", bufs=2) as xt_pool,
                tc.tile_pool(name="wk", bufs=2) as wk_pool,
                tc.tile_pool(name="ppk", bufs=2, space="PSUM") as ppk,
                tc.tile_pool(name="ppv", bufs=2, space="PSUM") as ppv,
            ):
                wv_sb = wv_pool.tile([P, 8, D], fr)
                nc.sync.dma_start(wv_sb[:], wv3)
                for half in range(2):
                    xts = []
                    for s2 in range(2):
                        s = half * 2 + s2
                        t = xt_pool.tile([P, 8, 512], fr, tag="xt", name=f"xt{s}")
                        nc.sync.dma_start(t[:], xT3[:, :, s * 512 : (s + 1) * 512])
                        xts.append(t)

                    for ec in range(8):
                        wk_ec = wk_pool.tile([P, 8, P], fr, tag="wk")
                        nc.sync.dma_start(
                            wk_ec[:], wk3[:, :, ec * P : (ec + 1) * P]
                        )
                        pss = [
                            ppk.tile([P, 512], f32, tag="ppk", name="ppk")
                            for _ in range(2)
                        ]
                        for do in range(8):
                            for s2 in range(2):
                                nc.tensor.matmul(
                                    pss[s2][:],
                                    wk_ec[:, do, :],
                                    xts[s2][:, do, :],
                                    start=(do == 0),
                                    stop=(do == 7),
                                )
                        for s2 in range(2):
                            nc.vector.tensor_copy(
                                out=kt_slabs[half * 2 + s2][:, ec, :],
                                in_=pss[s2][:],
                            )

                    for s2 in range(2):
                        s = half * 2 + s2
                        for kq in range(4):
                            pvs = [
                                ppv.tile([P, 512], f32, tag="ppv", name="ppv")
                                for _ in range(2)
                            ]
                            for do in range(8):
                                for es in range(2):
                                    nc.tensor.matmul(
                                        pvs[es][:],
                                        xts[s2][:, do, kq * P : (kq + 1) * P],
                                        wv_sb[:, do, es * 512 : (es + 1) * 512],
                                        start=(do == 0),
                                        stop=(do == 7),
                                    )
                            for es in range(2):
                                nc.vector.tensor_copy(
                                    out=v_slabs[s][:, kq, es * 512 : (es + 1) * 512],
                                    in_=pvs[es][:],
                                )

            # ---- phase C: Q^T projection (this core's 1024 queries) ----
            with (
                tc.tile_pool(name="wq", bufs=1) as wq_pool,
                tc.tile_pool(name="xq", bufs=1) as xq_pool,
                tc.tile_pool(name="ppq", bufs=4, space="PSUM") as ppq,
            ):
                wq_sb = wq_pool.tile([P, 8, D], fr)
                nc.sync.dma_start(wq_sb[:], wq3)
                xq_sb = xq_pool.tile([P, 8, NQ], fr)
                nc.sync.dma_start(xq_sb[:], xq3)
                for ec in range(8):
                    pqs = [ppq.tile([P, 512], f32, tag="ppq", name="ppq") for _ in range(2)]
                    for do in range(8):
                        for qs in range(2):
                            nc.tensor.matmul(
                                pqs[qs][:],
                                wq_sb[:, do, ec * P : (ec + 1) * P],
                                xq_sb[:, do, qs * 512 : (qs + 1) * 512],
                                start=(do == 0),
                                stop=(do == 7),
                            )
                    for qs in range(2):
                        nc.vector.tensor_copy(
                            out=qt[:, ec, qs * 512 : (qs + 1) * 512], in_=pqs[qs][:]
                        )

            # ---- phase D: scores + softmax + AV, per 256-query block ----
            with (
                tc.tile_pool(name="pt", bufs=2) as pt_pool,
                tc.tile_pool(name="ps", bufs=2, space="PSUM") as ps_pool,
                tc.tile_pool(name="po", bufs=4, space="PSUM") as po_pool,
                tc.tile_pool(name="pd", bufs=2, space="PSUM") as pd_pool,
                tc.tile_pool(name="ob", bufs=3) as ob_pool,
                tc.tile_pool(name="rc", bufs=4) as rc_pool,
            ):
                for i in range(4):
                    nk = 4 * i + 4  # causal: key chunks 0..nk-1
                    pt = pt_pool.tile([P, 16, QT], bf, tag="pt")
                    for kc in range(nk):
                        s, kq = kc // 4, kc % 4
                        ps = ps_pool.tile([P, QT], f32, tag="ps")
                        for ec in range(8):
                            nc.tensor.matmul(
                                ps[:],
                                kt_slabs[s][:, ec, kq * P : (kq + 1) * P],
                                qt[:, ec, i * QT : (i + 1) * QT],
                                start=(ec == 0),
                                stop=(ec == 7),
                            )
                        m = kc - 4 * i
                        if m >= 0:
                            nc.vector.tensor_add(
                                out=ps[:], in0=ps[:], in1=mask_sb[:, m, :]
                            )
                        nc.scalar.activation(
                            out=pt[:, kc, :], in_=ps[:], func=EXP, scale=SCALE
                        )
                    for qc in range(2):
                        dn = pd_pool.tile([P, 1], f32, tag="dn")
                        pos = [po_pool.tile([P, 512], f32, tag="po", name="po") for _ in range(2)]
                        for kc in range(nk):
                            s, kq = kc // 4, kc % 4
                            lhsT = pt[:, kc, qc * P : (qc + 1) * P]
                            first, last = kc == 0, kc == nk - 1
                            for es in range(2):
                                nc.tensor.matmul(
                                    pos[es][:],
                                    lhsT,
                                    v_slabs[s][:, kq, es * 512 : (es + 1) * 512],
                                    start=first,
                                    stop=last,
                                )
                            nc.tensor.matmul(
                                dn[:], lhsT, ones_sb[:], start=first, stop=last
                            )
                        rc = rc_pool.tile([P, 1], f32, tag="rc")
                        nc.vector.reciprocal(out=rc[:], in_=dn[:])
                        q0 = i * QT + qc * P
                        for es in range(2):
                            ob = ob_pool.tile([P, 512], f32, tag="ob")
                            nc.vector.tensor_scalar_mul(
                                out=ob[:], in0=pos[es][:], scalar1=rc[:]
                            )
                            nc.sync.dma_start(
                                out_ap[q0 : q0 + P, es * 512 : (es + 1) * 512], ob[:]
                            )

    nc.compile()
    return nc


def _get_nc():
    if "nc" not in _cache:
        _cache["nc"] = _build()
    return _cache["nc"]


def _host_masks(h: int) -> np.ndarray:
    # mask[m, p, j]: score tile at key chunk kc = 4*i+m, query block i.
    # global k = 512*i + 128*m + p, global q = 2*(256*i + j) + h.
    # keep (0.0) iff k <= q  <=>  p - 2j <= h - 128*m, else -1e30.
    m = np.arange(4)[:, None, None]
    p = np.arange(P)[None, :, None]
    j = np.arange(QT)[None, None, :]
    keep = (p - 2 * j) <= (h - 128 * m)
    return np.where(keep, np.float32(0.0), np.float32(NEG)).astype(np.float32)


def make_in_maps(x, Wq, Wk, Wv):
    wqT = np.ascontiguousarray(Wq.T).astype(np.float32, copy=False)
    wkT = np.ascontiguousarray(Wk.T).astype(np.float32, copy=False)
    wvT = np.ascontiguousarray(Wv.T).astype(np.float32, copy=False)
    masks_h = [_host_masks(0), _host_masks(1)]
    in_maps = []
    for c in range(N_CORES):
        b, h = c // 2, c % 2
        xb = np.asarray(x[b], dtype=np.float32)
        in_maps.append(
            {
                "xT": np.ascontiguousarray(xb.T),
                "xqT": np.ascontiguousarray(xb[h::2, :].T),
                "wqT": wqT,
                "wkT": wkT,
                "wvT": wvT,
                "masks": masks_h[h],
            }
        )
    return in_maps


def kernel(x, Wq, Wk, Wv):
    from concourse.bass_utils import run_bass_kernel_spmd

    nc = _get_nc()
    in_maps = make_in_maps(x, Wq, Wk, Wv)
    res = run_bass_kernel_spmd(nc, in_maps, core_ids=list(range(N_CORES)))
    out = np.empty((B, S, D), dtype=np.float32)
    for c in range(N_CORES):
        b, h = c // 2, c % 2
        out[b, h::2, :] = res.results[c]["out"]
    return out
